# revision 1
# baseline (speedup 1.0000x reference)
"""ComplexDenseSO2 Trainium2 kernel.

Computes out = (X @ conj(B)^T * w) @ B for complex X [64, 32400],
B [2048, 32400], w [2048], given as separate re/im fp32 planes.

Strategy (tensor-parallel over D across 8 cores):
  - Fold w into the first-matmul operand on the host:
    M = diag(w) @ conj(B), so mm1 output IS Y = X @ M^T.
  - Pad D 32400 -> 32768; core c owns d-slice [c*4096, (c+1)*4096).
  - mm1 (per core): stationary [Xr|Xi]^T d-tiles [128,128], moving
    M^T d-tiles [128, 512k]; PSUM accumulates over d -> partial Y
    in [j=128, k] layout (j: 0:64 real-X rows, 64:128 imag-X rows).
  - PE-transpose to k-major, combine re/im parts (free-dim slices),
    DMA to a DRAM bounce -> AllReduce(add) over the 8 cores (1 MB).
  - Post-AR: build fp16 stationaries YtA = [Yr|Yi], YtB = [-Yi|Yr].
  - mm2: out[128, d] PSUM accumulates YtA^T@Br' + YtB^T@Bi' over k,
    which yields rows 0:64 = Or, rows 64:128 = Oi directly.
  - fp16 operands use power-of-2 prescales (M*1024, B*256) to stay
    clear of fp16 subnormals; the epilogue descales by 2^-18.
"""

import sys

if "/opt/trn_rl_repo" not in sys.path:
    sys.path.insert(0, "/opt/trn_rl_repo")

import numpy as np

B_, K, D = 64, 2048, 32400
NCORES = 8
DP = 32768
DL = DP // NCORES  # 4096

COMPUTE_DT = "float16"  # or "bfloat16"
SCALE_M = 1024.0
SCALE_B = 256.0

_nc_cache = {}


def build_nc(n_cores=NCORES, k=K, dl=DL):
    import concourse.mybir as mybir
    from concourse import bacc
    import concourse.tile as tile
    from concourse.masks import make_identity

    fp = getattr(mybir.dt, COMPUTE_DT)
    f32 = mybir.dt.float32

    ndt = dl // 128  # mm1 d-tiles
    nkc = k // 512   # mm1 k-chunks
    nkb = k // 128   # k-blocks
    ndc = dl // 512  # mm2 d-chunks

    nc = bacc.Bacc(
        trn_type="TRN2",
        target_bir_lowering=False,
        debug=False,
        num_devices=n_cores,
    )
    xt = nc.dram_tensor("xt", [dl, 128], fp, kind="ExternalInput")
    mtr = nc.dram_tensor("mtr", [dl, k], fp, kind="ExternalInput")
    mti = nc.dram_tensor("mti", [dl, k], fp, kind="ExternalInput")
    bnr = nc.dram_tensor("bnr", [k, dl], fp, kind="ExternalInput")
    bni = nc.dram_tensor("bni", [k, dl], fp, kind="ExternalInput")
    out = nc.dram_tensor("out", [128, dl], f32, kind="ExternalOutput")

    with tile.TileContext(nc) as tc:
        with (
            tc.tile_pool(name="sb", bufs=2) as sb,
            tc.tile_pool(name="sbx", bufs=1) as sbx,
            tc.tile_pool(name="ps", bufs=1, space="PSUM") as ps,
            tc.tile_pool(name="dram", bufs=1, space="DRAM") as dram,
        ):
            ident = sbx.tile([128, 128], f32, tag="ident")
            make_identity(nc, ident)

            xts_all = sbx.tile([128, dl], fp, tag="xts_all", name="xts_all")
            nc.sync.dma_start(
                out=xts_all.rearrange("p (t j) -> p t j", j=128),
                in_=xt.ap().rearrange("(t p) j -> p t j", p=128),
            )
            xts = [xts_all[:, dt * 128 : (dt + 1) * 128] for dt in range(ndt)]

            arin = dram.tile([k, 128], fp, tag="arin", name="arin")
            arout = dram.tile(
                [k, 128], fp, tag="arout", name="arout", addr_space="Shared"
            )

            # ---------------- mm1 ----------------
            # Two passes over d, each covering a 1024-wide k-chunk pair.
            # Per d-tile one 256KB DMA per component; 4 matmuls share the
            # stationary x-tile. 4 PSUM accumulator banks per pass.
            kw = min(1024, k)
            nq = kw // 512
            for half in range(k // kw):
                ks = slice(half * kw, (half + 1) * kw)
                accs = []
                for q in range(nq):
                    a_r = ps.tile([128, 512], f32, tag=f"a{q}r", name=f"a{q}r")
                    a_i = ps.tile([128, 512], f32, tag=f"a{q}i", name=f"a{q}i")
                    accs.append((a_r, a_i))
                for dt in range(ndt):
                    rs = slice(dt * 128, (dt + 1) * 128)
                    mr_t = sb.tile([128, kw], fp, tag="mr", name="mr", bufs=10)
                    nc.sync.dma_start(out=mr_t, in_=mtr[rs, ks])
                    mi_t = sb.tile([128, kw], fp, tag="mi", name="mi", bufs=10)
                    nc.sync.dma_start(out=mi_t, in_=mti[rs, ks])
                    st, sp = dt == 0, dt == ndt - 1
                    for q in range(nq):
                        qs = slice(q * 512, (q + 1) * 512)
                        nc.tensor.matmul(accs[q][0], lhsT=xts[dt], rhs=mr_t[:, qs], start=st, stop=sp)
                        nc.tensor.matmul(accs[q][1], lhsT=xts[dt], rhs=mi_t[:, qs], start=st, stop=sp)
                for q in range(nq):
                    pscr = sb.tile([128, 512], f32, tag="pscr", name="pscr", bufs=2)
                    nc.vector.tensor_copy(pscr, accs[q][0])
                    psci = sb.tile([128, 512], f32, tag="psci", name="psci", bufs=2)
                    nc.scalar.copy(psci, accs[q][1])
                    for j in range(4):
                        kb = half * (4 * nq) + q * 4 + j
                        js = slice(j * 128, (j + 1) * 128)
                        tp_r = ps.tile([128, 128], f32, tag="tp_r", name="tp_r", bufs=1)
                        nc.tensor.transpose(tp_r, pscr[:, js], ident)
                        tp_i = ps.tile([128, 128], f32, tag="tp_i", name="tp_i", bufs=1)
                        nc.tensor.transpose(tp_i, psci[:, js], ident)
                        # DVE tensor_tensor may read only one PSUM input:
                        # evacuate tp_r to SBUF, combine against tp_i in PSUM.
                        cc_r = sb.tile([128, 128], f32, tag="cc_r", name="cc_r", bufs=3)
                        nc.vector.tensor_copy(cc_r, tp_r)
                        c_kb = sb.tile([128, 128], fp, tag="c_kb", name="c_kb", bufs=4)
                        # Yr = re(X@Mr^T) - im(X@Mi^T); Yi = im(X@Mr^T) + re(X@Mi^T)
                        nc.vector.tensor_sub(c_kb[:, 0:64], cc_r[:, 0:64], tp_i[:, 64:128])
                        nc.vector.tensor_add(c_kb[:, 64:128], cc_r[:, 64:128], tp_i[:, 0:64])
                        nc.sync.dma_start(out=arin[kb * 128 : (kb + 1) * 128, :], in_=c_kb)

            # ---------------- AllReduce ----------------
            nc.gpsimd.collective_compute(
                "AllReduce",
                mybir.AluOpType.add,
                ins=[arin.opt()],
                outs=[arout.opt()],
                replica_groups=[list(range(n_cores))],
            )

            # ---------------- build mm2 stationaries ----------------
            # AR output is Y*SCALE_M in fp16; use it directly as the
            # stationary (the epilogue descales by SCALE_M*SCALE_B).
            ytA, ytB = [], []
            for kb in range(nkb):
                a_t = sbx.tile([128, 128], fp, tag=f"ytA{kb}", name=f"ytA{kb}")
                nc.sync.dma_start(out=a_t, in_=arout[kb * 128 : (kb + 1) * 128, :])
                b_t = sbx.tile([128, 128], fp, tag=f"ytB{kb}", name=f"ytB{kb}")
                nc.vector.tensor_scalar_mul(b_t[:, 0:64], a_t[:, 64:128], -1.0)
                nc.vector.tensor_copy(b_t[:, 64:128], a_t[:, 0:64])
                ytA.append(a_t)
                ytB.append(b_t)

            # ---------------- mm2 ----------------
            for pr in range(ndc // 2):
                dc0, dc1 = 2 * pr, 2 * pr + 1
                s0 = slice(dc0 * 512, (dc0 + 1) * 512)
                s1 = slice(dc1 * 512, (dc1 + 1) * 512)
                sp_pair = slice(dc0 * 512, (dc0 + 2) * 512)
                po0 = ps.tile([128, 512], f32, tag="po0", name="po0")
                po1 = ps.tile([128, 512], f32, tag="po1", name="po1")
                for kb in range(nkb):
                    rs = slice(kb * 128, (kb + 1) * 128)
                    r01 = sb.tile([128, 1024], fp, tag="bnr01", name="bnr01", bufs=8)
                    nc.sync.dma_start(out=r01, in_=bnr[rs, sp_pair])
                    i01 = sb.tile([128, 1024], fp, tag="bni01", name="bni01", bufs=8)
                    nc.sync.dma_start(out=i01, in_=bni[rs, sp_pair])
                    st, sp = kb == 0, kb == nkb - 1
                    nc.tensor.matmul(po0, lhsT=ytA[kb], rhs=r01[:, 0:512], start=st, stop=False)
                    nc.tensor.matmul(po1, lhsT=ytA[kb], rhs=r01[:, 512:1024], start=st, stop=False)
                    nc.tensor.matmul(po0, lhsT=ytB[kb], rhs=i01[:, 0:512], start=False, stop=sp)
                    nc.tensor.matmul(po1, lhsT=ytB[kb], rhs=i01[:, 512:1024], start=False, stop=sp)
                o0 = sb.tile([128, 512], f32, tag="o0", name="o0", bufs=2)
                nc.vector.tensor_scalar_mul(o0, po0, 1.0 / (SCALE_M * SCALE_B))
                nc.sync.dma_start(out=out[:, s0], in_=o0)
                o1 = sb.tile([128, 512], f32, tag="o1", name="o1", bufs=2)
                nc.vector.tensor_scalar_mul(o1, po1, 1.0 / (SCALE_M * SCALE_B))
                nc.sync.dma_start(out=out[:, s1], in_=o1)

    nc.compile()
    return nc


def _get_nc(n_cores=NCORES, k=K, dl=DL):
    key = (n_cores, k, dl)
    if key not in _nc_cache:
        _nc_cache[key] = build_nc(n_cores, k, dl)
    return _nc_cache[key]


def _prep_in_maps(X_re, X_im, bases_re, bases_im, weight_re, weight_im):
    cdt = np.float16 if COMPUTE_DT == "float16" else None
    if cdt is None:
        import ml_dtypes

        cdt = ml_dtypes.bfloat16

    f32 = np.float32
    X_re = np.asarray(X_re, f32)
    X_im = np.asarray(X_im, f32)
    bases_re = np.asarray(bases_re, f32)
    bases_im = np.asarray(bases_im, f32)
    wr = np.asarray(weight_re, f32)[:, None]
    wi = np.asarray(weight_im, f32)[:, None]

    # M = diag(w) @ conj(B): Mr = wr*Br + wi*Bi ; Mi = wi*Br - wr*Bi
    mr = (wr * bases_re + wi * bases_im) * np.float32(SCALE_M)
    mi = (wi * bases_re - wr * bases_im) * np.float32(SCALE_M)
    bsr = bases_re * np.float32(SCALE_B)
    bsi = bases_im * np.float32(SCALE_B)

    in_maps = []
    for c in range(NCORES):
        lo = c * DL
        hi = min((c + 1) * DL, D)
        n = hi - lo
        xt = np.zeros((DL, 128), cdt)
        if n > 0:
            xt[:n, 0:64] = X_re[:, lo:hi].T.astype(cdt)
            xt[:n, 64:128] = X_im[:, lo:hi].T.astype(cdt)
        mtr = np.zeros((DL, K), cdt)
        mti = np.zeros((DL, K), cdt)
        bnr = np.zeros((K, DL), cdt)
        bni = np.zeros((K, DL), cdt)
        if n > 0:
            mtr[:n, :] = mr[:, lo:hi].T.astype(cdt)
            mti[:n, :] = mi[:, lo:hi].T.astype(cdt)
            bnr[:, :n] = bsr[:, lo:hi].astype(cdt)
            bni[:, :n] = bsi[:, lo:hi].astype(cdt)
        in_maps.append({"xt": xt, "mtr": mtr, "mti": mti, "bnr": bnr, "bni": bni})
    return in_maps


def run(inputs, trace=False, trace_kwargs=None):
    """Returns (full complex64 output [64, 32400], BassKernelResults)."""
    from concourse.bass_utils import run_bass_kernel_spmd

    in_maps = _prep_in_maps(**inputs)
    nc = _get_nc()
    res = run_bass_kernel_spmd(
        nc,
        in_maps,
        core_ids=list(range(NCORES)),
        trace=trace,
        **(trace_kwargs or {}),
    )
    parts = []
    for c in range(NCORES):
        o = res.results[c]["out"]
        parts.append(o[0:64, :] + 1j * o[64:128, :].astype(np.complex64))
    full = np.concatenate(parts, axis=1)[:, :D].astype(np.complex64)
    return full, res


def kernel(**inputs) -> np.ndarray:
    out, _ = run(inputs, trace=False)
    return out

